# revision 2
# baseline (speedup 1.0000x reference)
"""AdmissibleStatesHead on 8 Trainium2 NeuronCores.

marginals[c] = segment_sum(softmax(E @ W.T + b), digit_c)  ==  P @ M_c
where M is a one-hot [N_VALID, 48] matrix built on host from valid_states.

Device work per core (valid-states sharded 8 ways, batch replicated):
  logits^T tile [128v, 512b] = sum_k wt[k,v].T @ et[k,b]   (fp8 DoubleRow PE, fp32 PSUM)
  exp tile = Exp(logits^T * inv_scale + bias)              (ScalarE, PSUM -> SBUF fp8)
  U^T [48, 512b] += M_chunk.T @ exp_tile                   (fp8 DoubleRow PE)
Host: sum per-core partials, normalize by concept-0 bucket sum (= softmax
denominator), reshape to [6, B, 8]. W is pre-scaled by a power of two into
fp8's range; the Exp activation's free affine undoes it.

Schedule notes (from trace analysis): the PE matmul stream is gapless at
216ns/MM once the clock gate opens (~5.45us after first PE activity), so the
wins are at the edges: (1) input DMA triggers cost ~0.6us of engine time each
and are split across both HWDGE engines (SP + Activation) so the first real
matmul's data lands ~7.7us instead of ~8.4; (2) the clock-gate warm-up runs
tiny N=128 matmuls on uninitialized SBUF (no memset dependency) and hands off
to real matmuls as soon as DMA lands, instead of grinding 10 fixed N=512
throwaways; (3) the last batch tile's final exp/MM2/copy are split into
column halves across engines to shorten the serial drain tail.

Set KERNEL_BF16=1 for a bf16 fallback (~1.8x slower, ~15x more accurate);
KERNEL_TRACE=1 captures an NTFF profile and fills LAST_EXEC_NS.
"""

import os
import sys
import types

import numpy as np
import ml_dtypes

OUTCOMES = [8, 8, 8, 8, 8, 8]
N_TOTAL = 262144
N_VALID = 8192
B, D = 4096, 1024
N_CORES = 8
P = 128
V_S = N_VALID // N_CORES  # 1024 valid states per core
NK = D // P               # 8 contraction chunks
NV = V_S // P             # 8 v-tiles per core
NB = B // 512             # 8 batch tiles of 512
NJ = 48                   # 6 concepts x 8 outcomes

# W values are small (~N(0, 0.02^2) per spec); scale into fp8e4m3's normal
# range and undo the scale for free inside the Exp activation. Chosen per
# call from the data as a power of two; the compiled module is cached per
# scale value.
DEFAULT_W_SCALE = 64.0

USE_BF16 = bool(os.environ.get("KERNEL_BF16"))
N_WARM = int(os.environ.get("KERNEL_WARM", "14"))

LAST_EXEC_NS = None
LAST_RESULT = None
_compiled_cache = {}


def _pick_w_scale(wmax):
    import math

    if not np.isfinite(wmax) or wmax <= 0:
        return DEFAULT_W_SCALE
    # keep max|W*scale| around <=192 (fp8e4m3 max 448), scale a power of 2
    s = 2.0 ** math.floor(math.log2(192.0 / wmax))
    return float(min(max(s, 2.0 ** -10), 2.0 ** 20))


def _split_excess_waits(nc, limit=1):
    """This walrus build rejects instructions carrying more than ~1 sync-wait
    ("Too many sync wait commands"). Hoist excess waits onto injected NoOps
    right before the instruction on the same engine — sequencers are in-order,
    so the semantics are identical."""
    import concourse.mybir as mybir

    ctr = 0
    main_bb = nc.m.functions[0].blocks[0]
    stripped = []
    for ins in main_bb.instructions:
        nm = str(ins.name)
        op = ins.concise_opcode()
        if op == "Drain" or (op == "EventSemaphore" and nm.startswith("barrier_")):
            continue
        stripped.append(ins)
    main_bb.instructions = stripped
    for fn in nc.m.functions:
        for bb in fn.blocks:
            insts = bb.instructions
            new = []
            changed = False
            for ins in insts:
                si = ins.sync_info
                lim = 1 if ins.concise_opcode() == "Drain" else limit
                if si is not None and len(si.on_wait) > lim:
                    waits = list(si.on_wait)
                    for w in waits[:-lim]:
                        ctr += 1
                        nop = mybir.InstNoOp(name=f"waitsplit_{ctr}", ins=[], outs=[])
                        nop.engine = ins.engine
                        nop.sync_info = mybir.SyncInfo(on_update=[], on_wait=[w])
                        new.append(nop)
                    ins.sync_info = mybir.SyncInfo(
                        on_update=list(si.on_update), on_wait=waits[-lim:]
                    )
                    changed = True
                new.append(ins)
            if changed:
                bb.instructions = new


def _patch_tile_tail():
    import concourse.tile as tile
    from concourse.vector_clock import ScopedClock

    if getattr(tile.TileContext, "_tail_patched", False):
        return

    def _drain_and_barrier(self, tick_clock, wait_clock):
        drain_inst = self.nc.sync.drain()
        wait_clock.add_sem_waits(
            drain_inst.ins, ScopedClock({None: tick_clock.global_clock})
        )
        self.nc.all_engine_barrier()
        popped = self.nc._tile_sem_poison_stack.pop()
        assert popped is self._sem_poison
        self.nc.clear_and_free_semaphores(list(self.sems.allocated().values()))

    tile.TileContext._drain_and_barrier = _drain_and_barrier
    tile.TileContext._tail_patched = True


def _build_nc(w_scale):
    import concourse.bass as bass
    import concourse.mybir as mybir
    import concourse.tile as tile

    _patch_tile_tail()

    f32 = mybir.dt.float32
    bf16 = mybir.dt.bfloat16
    fp8 = mybir.dt.float8e4
    Exp = mybir.ActivationFunctionType.Exp
    Copy = mybir.ActivationFunctionType.Copy

    in_dt = bf16 if USE_BF16 else fp8
    exp_scale = 1.0 if USE_BF16 else 1.0 / w_scale

    nc = bass.Bass()
    wt = nc.dram_tensor("wt", [P, NV, NK, P], in_dt, kind="ExternalInput")
    et = nc.dram_tensor("et", [NB, P, NK, 512], in_dt, kind="ExternalInput")
    mm = nc.dram_tensor("mm", [P, NV, NJ], in_dt, kind="ExternalInput")
    bias = nc.dram_tensor("bias", [P, NV], f32, kind="ExternalInput")
    out = nc.dram_tensor("out", [NJ, NB, 512], f32, kind="ExternalOutput")

    # Clock-gate warm-up scratch: read uninitialized SBUF (fp8 garbage is
    # numerically irrelevant — the warm psum is never read) so the PE can
    # start the instant its preamble ends, with no memset dependency.
    warm_g = nc.alloc_sbuf_tensor("warm_garbage", [P, P], in_dt)
    warm_a = nc.alloc_sbuf_tensor("warm_act_out", [P, 16], in_dt)

    with (
        tile.TileContext(nc) as tc,
        tc.tile_pool(name="const", bufs=1) as cpool,
        tc.tile_pool(name="etp", bufs=8) as epool,
        tc.tile_pool(name="expp", bufs=3) as xpool,
        tc.tile_pool(name="ps", bufs=6, space="PSUM") as pspool,
        tc.tile_pool(name="ps2", bufs=2, space="PSUM") as ps2pool,
        tc.tile_pool(name="uo", bufs=2) as upool,
    ):
        # PE HAM warm-up: the clock gate only opens after ~5.4us of sustained
        # PE activity (measured: first-MM + 5.45us). Run tiny N=128 matmuls on
        # garbage SBUF from the earliest possible instant; the real matmul
        # stream takes over as soon as its DMA lands (~7.7us) and rides the
        # ramp to 2.4 GHz.
        warm_ps = pspool.tile([P, 512], f32, tag="ps")
        for _ in range(N_WARM):
            nc.tensor.matmul(
                warm_ps[:, 0:P],
                lhsT=warm_g[:],
                rhs=warm_g[:],
                start=True,
                stop=True,
            )

        # Input DMA triggers cost ~0.6us of engine time each; split them
        # across both HWDGE engines (SP=sync, Activation=scalar) so the first
        # matmul's operands (wt v=0 on sync, et0 k0:4 on scalar) land in
        # parallel. Later tiles are staged so each transfer completes well
        # before its consumer at the measured 216ns/MM cadence.
        wt_sb = cpool.tile([P, NV, NK, P], in_dt)
        nc.sync.dma_start(wt_sb[:, 0], wt[:, 0])
        et0a = cpool.tile([P, 4, 512], in_dt)
        nc.scalar.dma_start(et0a[:], et[0][:, 0:4])
        nc.sync.dma_start(wt_sb[:, 1], wt[:, 1])
        et0b = cpool.tile([P, 4, 512], in_dt)
        nc.scalar.dma_start(et0b[:], et[0][:, 4:8])
        nc.sync.dma_start(wt_sb[:, 2:8], wt[:, 2:8])
        b_sb = cpool.tile([P, NV], f32)
        nc.scalar.dma_start(b_sb[:], bias[:])
        # Exp table load (~1.3us) on the scalar engine, after its DMA
        # triggers but before the first real activation needs it.
        nc.scalar.activation(warm_a[:], warm_g[:, 0:16], Exp)

        et_tiles = {}

        def load_et(n, engine):
            t = epool.tile([P, NK, 512], in_dt, tag="et", name="et_t")
            engine.dma_start(t[:], et[n])
            et_tiles[n] = (t[:, 0:4], t[:, 4:8])

        et_tiles[0] = (et0a[:], et0b[:])
        load_et(1, nc.sync)
        load_et(2, nc.sync)
        load_et(4, nc.sync)
        load_et(6, nc.sync)

        # m_sb and the odd et tiles are triggered from the scalar engine
        # mid-stream (interleaved with n=0's activations) — emitted inside
        # the main loop below so they sit at the right point of the
        # in-order Activation instruction stream.
        m_sb = cpool.tile([P, NV, NJ], in_dt)

        def emit_mm2(n, exp_t):
            last = n == NB - 1
            ups = ps2pool.tile([NJ, 512], f32, tag="ups")
            if USE_BF16:
                for v in range(NV):
                    nc.tensor.matmul(
                        ups[:],
                        lhsT=m_sb[:, v, :],
                        rhs=exp_t[:, v, :],
                        start=(v == 0),
                        stop=(v == NV - 1),
                    )
                u_sb = upool.tile([NJ, 512], f32, tag="u")
                nc.vector.tensor_copy(u_sb[:], ups[:])
            else:
                for v in range(0, NV - 2, 2):
                    nc.tensor.matmul(
                        ups[:],
                        lhsT=m_sb[:, v : v + 2, :],
                        rhs=exp_t[:, v : v + 2, :],
                        start=(v == 0),
                        stop=False,
                        perf_mode=mybir.MatmulPerfMode.DoubleRow,
                    )
                u_sb = upool.tile([NJ, 512], f32, tag="u")
                if last:
                    # Final drain chain: split the last v-pair MM2 (and the
                    # psum->sbuf copy) into column halves so the copy of the
                    # first half overlaps the matmul of the second, across
                    # Vector and Scalar.
                    for h0, h1, eng, fn in (
                        (0, 256, "vec", None),
                        (256, 512, "act", None),
                    ):
                        nc.tensor.matmul(
                            ups[:, h0:h1],
                            lhsT=m_sb[:, NV - 2 : NV, :],
                            rhs=exp_t[:, NV - 2 : NV, h0:h1],
                            start=False,
                            stop=True,
                            perf_mode=mybir.MatmulPerfMode.DoubleRow,
                        )
                        if eng == "vec":
                            nc.vector.tensor_copy(u_sb[:, h0:h1], ups[:, h0:h1])
                        else:
                            nc.scalar.activation(u_sb[:, h0:h1], ups[:, h0:h1], Copy)
                else:
                    nc.tensor.matmul(
                        ups[:],
                        lhsT=m_sb[:, NV - 2 : NV, :],
                        rhs=exp_t[:, NV - 2 : NV, :],
                        start=False,
                        stop=True,
                        perf_mode=mybir.MatmulPerfMode.DoubleRow,
                    )
                    nc.vector.tensor_copy(u_sb[:], ups[:])
            nc.sync.dma_start(out[:, n, :], u_sb[:])

        pending = None  # (n, exp_t) whose MM2 is deferred one tile
        for n in range(NB):
            et_a, et_b = et_tiles[n]
            exp_t = xpool.tile([P, NV, 512], in_dt, tag="exp")
            last = n == NB - 1
            for v in range(NV):
                ps = pspool.tile([P, 512], f32, tag="ps")
                if USE_BF16:
                    for k in range(NK):
                        rhs = et_a[:, k, :] if k < 4 else et_b[:, k - 4, :]
                        nc.tensor.matmul(
                            ps[:],
                            lhsT=wt_sb[:, v, k],
                            rhs=rhs,
                            start=(k == 0),
                            stop=(k == NK - 1),
                        )
                else:
                    for k in range(0, NK, 2):
                        rhs = (
                            et_a[:, k : k + 2, :]
                            if k < 4
                            else et_b[:, k - 4 : k - 2, :]
                        )
                        nc.tensor.matmul(
                            ps[:],
                            lhsT=wt_sb[:, v, k : k + 2],
                            rhs=rhs,
                            start=(k == 0),
                            stop=(k == NK - 2),
                            perf_mode=mybir.MatmulPerfMode.DoubleRow,
                        )
                if last and v == NV - 1:
                    # split the final exp so MM2's last half-columns can
                    # start after only half the activation
                    nc.scalar.activation(
                        exp_t[:, v, 0:256],
                        ps[:, 0:256],
                        Exp,
                        bias=b_sb[:, v : v + 1],
                        scale=exp_scale,
                    )
                    nc.scalar.activation(
                        exp_t[:, v, 256:512],
                        ps[:, 256:512],
                        Exp,
                        bias=b_sb[:, v : v + 1],
                        scale=exp_scale,
                    )
                else:
                    nc.scalar.activation(
                        exp_t[:, v, :],
                        ps[:],
                        Exp,
                        bias=b_sb[:, v : v + 1],
                        scale=exp_scale,
                    )
                if n == 0:
                    # mid-stream scalar-engine DMA triggers (see above)
                    if v == 1:
                        nc.scalar.dma_start(m_sb[:], mm[:])
                    elif v == 3:
                        load_et(3, nc.scalar)
                    elif v == 5:
                        load_et(5, nc.scalar)
                    elif v == 7:
                        load_et(7, nc.scalar)
                if pending is not None and v == 1:
                    emit_mm2(*pending)
                    pending = None
            pending = (n, exp_t)
        emit_mm2(*pending)
    _split_excess_waits(nc)
    return nc


def _install_ntff_hook():
    """bass_utils' axon trace path imports antenv.axon_hooks, absent in this
    image; shim it using trn_boot's ctypes NTFF hook."""
    if "antenv.axon_hooks" in sys.modules:
        return
    try:
        from trn_agent_boot.trn_boot import _ntff_profile_via_ctypes

        hook = _ntff_profile_via_ctypes("/opt/axon/libaxon_pjrt.so")
    except Exception:
        hook = None
    mod = types.ModuleType("antenv.axon_hooks")
    mod.get_axon_ntff_profile_hook = lambda: hook
    sys.modules["antenv.axon_hooks"] = mod


def kernel(embeddings, W, b, valid_states):
    global LAST_EXEC_NS, LAST_RESULT
    E = np.asarray(embeddings, dtype=np.float32)
    Wf = np.asarray(W, dtype=np.float32)
    bf = np.asarray(b, dtype=np.float32)
    vs = np.asarray(valid_states).astype(np.int64)

    bf16 = ml_dtypes.bfloat16
    if USE_BF16:
        in_dt = bf16
        Wp = Wf
        w_scale = 1.0
    else:
        in_dt = ml_dtypes.float8_e4m3
        w_scale = _pick_w_scale(float(np.abs(Wf).max()))
        Wp = Wf * w_scale

    # et[n, p, k, j] = E[n*512+j, k*128+p]
    Et = E.T.astype(in_dt)  # [D, B]
    et_host = np.ascontiguousarray(Et.reshape(NK, P, NB, 512).transpose(2, 1, 0, 3))

    # One-hot segment matrix M [N_VALID, 48]
    M = np.zeros((N_VALID, NJ), dtype=in_dt)
    stride = N_TOTAL
    for c, n_i in enumerate(OUTCOMES):
        stride //= n_i
        digit = (vs // stride) % n_i
        M[np.arange(N_VALID), c * 8 + digit] = 1

    in_maps = []
    for core in range(N_CORES):
        sl = slice(core * V_S, (core + 1) * V_S)
        wt_host = np.ascontiguousarray(
            Wp[sl, :].T.astype(in_dt).reshape(NK, P, NV, P).transpose(1, 2, 0, 3)
        )
        m_host = np.ascontiguousarray(M[sl].reshape(NV, P, NJ).transpose(1, 0, 2))
        b_host = np.ascontiguousarray(bf[sl].reshape(NV, P).T)
        in_maps.append({"wt": wt_host, "et": et_host, "mm": m_host, "bias": b_host})

    from concourse.bass_utils import run_bass_kernel_spmd

    key = (USE_BF16, w_scale)
    if key not in _compiled_cache:
        _compiled_cache[key] = _build_nc(w_scale)
    nc_mod = _compiled_cache[key]

    kwargs = {}
    if os.environ.get("KERNEL_TRACE"):
        _install_ntff_hook()
        kwargs["trace"] = True

    res = run_bass_kernel_spmd(
        nc_mod, in_maps, core_ids=list(range(N_CORES)), **kwargs
    )
    LAST_EXEC_NS = res.exec_time_ns
    LAST_RESULT = res

    U = np.zeros((NJ, B), dtype=np.float64)
    for r in res.results:
        U += r["out"].reshape(NJ, B).astype(np.float64)
    denom = U[0:8].sum(axis=0)  # [B] total softmax denominator
    marg = U.reshape(6, 8, B) / denom  # [6, 8, B]
    return np.ascontiguousarray(marg.transpose(0, 2, 1)).astype(np.float32)


# revision 7
# speedup vs baseline: 1.0147x; 1.0147x over previous
"""AdmissibleStatesHead on 8 Trainium2 NeuronCores.

marginals[c] = segment_sum(softmax(E @ W.T + b), digit_c)  ==  P @ M_c
where M is a one-hot [N_VALID, 48] matrix built on host from valid_states.

Device work per core (valid-states sharded 8 ways, batch replicated):
  logits^T tile [128v, 512b] = sum_k wt[k,v].T @ et[k,b]   (fp8 DoubleRow PE, fp32 PSUM)
  exp tile = Exp(logits^T * inv_scale + bias)              (ScalarE, PSUM -> SBUF fp8)
  U^T [48, 512b] += M_chunk.T @ exp_tile                   (fp8 DoubleRow PE)
Host: sum per-core partials, normalize by concept-0 bucket sum (= softmax
denominator), reshape to [6, B, 8]. W is pre-scaled by a power of two into
fp8's range; the Exp activation's free affine undoes it.

Schedule notes (from trace analysis): the PE matmul stream is gapless at
216ns/MM once the clock gate opens (~5.45us after first PE activity), so the
wins are at the edges: (1) input DMA triggers cost ~0.6us of engine time each
and are split across both HWDGE engines (SP + Activation) so the first real
matmul's data lands ~7.7us instead of ~8.4; (2) the clock-gate warm-up runs
tiny N=128 matmuls on uninitialized SBUF (no memset dependency) and hands off
to real matmuls as soon as DMA lands, instead of grinding 10 fixed N=512
throwaways; (3) the last batch tile's final exp/MM2/copy are split into
column halves across engines to shorten the serial drain tail.

Set KERNEL_BF16=1 for a bf16 fallback (~1.8x slower, ~15x more accurate);
KERNEL_TRACE=1 captures an NTFF profile and fills LAST_EXEC_NS.
"""

import os
import sys
import types

import numpy as np
import ml_dtypes

OUTCOMES = [8, 8, 8, 8, 8, 8]
N_TOTAL = 262144
N_VALID = 8192
B, D = 4096, 1024
N_CORES = 8
P = 128
V_S = N_VALID // N_CORES  # 1024 valid states per core
NK = D // P               # 8 contraction chunks
NV = V_S // P             # 8 v-tiles per core
NB = B // 512             # 8 batch tiles of 512
NJ = 48                   # 6 concepts x 8 outcomes

# W values are small (~N(0, 0.02^2) per spec); scale into fp8e4m3's normal
# range and undo the scale for free inside the Exp activation. Chosen per
# call from the data as a power of two; the compiled module is cached per
# scale value.
DEFAULT_W_SCALE = 64.0

USE_BF16 = bool(os.environ.get("KERNEL_BF16"))
N_WARM = int(os.environ.get("KERNEL_WARM", "26"))

LAST_EXEC_NS = None
LAST_RESULT = None
_compiled_cache = {}


def _pick_w_scale(wmax):
    import math

    if not np.isfinite(wmax) or wmax <= 0:
        return DEFAULT_W_SCALE
    # keep max|W*scale| around <=192 (fp8e4m3 max 448), scale a power of 2
    s = 2.0 ** math.floor(math.log2(192.0 / wmax))
    return float(min(max(s, 2.0 ** -10), 2.0 ** 20))


def _split_excess_waits(nc, limit=1):
    """This walrus build rejects instructions carrying more than ~1 sync-wait
    ("Too many sync wait commands"). Hoist excess waits onto injected NoOps
    right before the instruction on the same engine — sequencers are in-order,
    so the semantics are identical."""
    import concourse.mybir as mybir

    ctr = 0
    main_bb = nc.m.functions[0].blocks[0]
    stripped = []
    for ins in main_bb.instructions:
        nm = str(ins.name)
        op = ins.concise_opcode()
        if op == "Drain" or (op == "EventSemaphore" and nm.startswith("barrier_")):
            continue
        stripped.append(ins)
    main_bb.instructions = stripped
    for fn in nc.m.functions:
        for bb in fn.blocks:
            insts = bb.instructions
            new = []
            changed = False
            for ins in insts:
                si = ins.sync_info
                lim = 1 if ins.concise_opcode() == "Drain" else limit
                if si is not None and len(si.on_wait) > lim:
                    waits = list(si.on_wait)
                    for w in waits[:-lim]:
                        ctr += 1
                        nop = mybir.InstNoOp(name=f"waitsplit_{ctr}", ins=[], outs=[])
                        nop.engine = ins.engine
                        nop.sync_info = mybir.SyncInfo(on_update=[], on_wait=[w])
                        new.append(nop)
                    ins.sync_info = mybir.SyncInfo(
                        on_update=list(si.on_update), on_wait=waits[-lim:]
                    )
                    changed = True
                new.append(ins)
            if changed:
                bb.instructions = new


def _patch_tile_tail():
    import concourse.tile as tile
    from concourse.vector_clock import ScopedClock

    if getattr(tile.TileContext, "_tail_patched", False):
        return

    def _drain_and_barrier(self, tick_clock, wait_clock):
        drain_inst = self.nc.sync.drain()
        wait_clock.add_sem_waits(
            drain_inst.ins, ScopedClock({None: tick_clock.global_clock})
        )
        self.nc.all_engine_barrier()
        popped = self.nc._tile_sem_poison_stack.pop()
        assert popped is self._sem_poison
        self.nc.clear_and_free_semaphores(list(self.sems.allocated().values()))

    tile.TileContext._drain_and_barrier = _drain_and_barrier
    tile.TileContext._tail_patched = True


def _build_nc(w_scale):
    import concourse.bass as bass
    import concourse.mybir as mybir
    import concourse.tile as tile

    _patch_tile_tail()

    f32 = mybir.dt.float32
    bf16 = mybir.dt.bfloat16
    fp8 = mybir.dt.float8e4
    Exp = mybir.ActivationFunctionType.Exp

    in_dt = bf16 if USE_BF16 else fp8
    exp_scale = 1.0 if USE_BF16 else 1.0 / w_scale

    nc = bass.Bass()
    wt = nc.dram_tensor("wt", [P, NV, NK, P], in_dt, kind="ExternalInput")
    et = nc.dram_tensor("et", [NB, P, NK, 512], in_dt, kind="ExternalInput")
    mm = nc.dram_tensor("mm", [P, NV, NJ], in_dt, kind="ExternalInput")
    bias = nc.dram_tensor("bias", [P, NV], f32, kind="ExternalInput")
    out = nc.dram_tensor("out", [NJ, NB, 512], f32, kind="ExternalOutput")

    # Clock-gate warm-up scratch: read uninitialized SBUF (fp8 garbage is
    # numerically irrelevant — the warm psum is never read) so the PE can
    # start the instant its preamble ends, with no memset dependency.
    warm_g = nc.alloc_sbuf_tensor("warm_garbage", [P, P], in_dt)
    warm_a = nc.alloc_sbuf_tensor("warm_act_out", [P, 16], in_dt)

    with (
        tile.TileContext(nc) as tc,
        tc.tile_pool(name="const", bufs=1) as cpool,
        tc.tile_pool(name="etp", bufs=8) as epool,
        tc.tile_pool(name="expp", bufs=3) as xpool,
        tc.tile_pool(name="ps", bufs=6, space="PSUM") as pspool,
        tc.tile_pool(name="ps2", bufs=2, space="PSUM") as ps2pool,
        tc.tile_pool(name="uo", bufs=2) as upool,
    ):
        # PE HAM warm-up: the clock gate only opens after ~5.4us of sustained
        # PE activity (measured: first-MM + 5.45us). Run tiny N=128 matmuls on
        # garbage SBUF from the earliest possible instant; the real matmul
        # stream takes over as soon as its DMA lands (~7.7us) and rides the
        # ramp to 2.4 GHz.
        warm_ps = pspool.tile([P, 512], f32, tag="ps")
        for _ in range(N_WARM):
            nc.tensor.matmul(
                warm_ps[:, 0:P],
                lhsT=warm_g[:],
                rhs=warm_g[:],
                start=True,
                stop=True,
            )

        # Input DMA triggers cost ~0.6us of engine time each; split them
        # across both HWDGE engines (SP=sync, Activation=scalar) so the first
        # matmul's operands land in parallel. Packet size drives DMA rate
        # (1KB rows ran ~100-150 GB/s, 4KB+ rows ~400 GB/s), so wt is pulled
        # in two 4KB-row halves rather than per-v 1KB-row slices.
        wt_sb = cpool.tile([P, NV, NK, P], in_dt)
        nc.sync.dma_start(wt_sb[:, 0:4], wt[:, 0:4])
        et0a = cpool.tile([P, 4, 512], in_dt)
        nc.scalar.dma_start(et0a[:], et[0][:, 0:4])
        nc.sync.dma_start(wt_sb[:, 4:8], wt[:, 4:8])
        et0b = cpool.tile([P, 4, 512], in_dt)
        nc.scalar.dma_start(et0b[:], et[0][:, 4:8])
        b_sb = cpool.tile([P, NV], f32)
        nc.scalar.dma_start(b_sb[:], bias[:])
        # Exp table load (~1.3us) on the scalar engine, after its DMA
        # triggers but before the first real activation needs it.
        nc.scalar.activation(warm_a[:], warm_g[:, 0:16], Exp)

        et_tiles = {}

        def load_et(n, engine):
            t = epool.tile([P, NK, 512], in_dt, tag="et", name="et_t")
            engine.dma_start(t[:], et[n])
            et_tiles[n] = (t[:, 0:4], t[:, 4:8])

        et_tiles[0] = (et0a[:], et0b[:])
        load_et(1, nc.sync)
        load_et(2, nc.sync)
        load_et(4, nc.sync)
        load_et(6, nc.sync)

        # m_sb and the odd et tiles are triggered from the scalar engine
        # mid-stream (interleaved with n=0's activations) — emitted inside
        # the main loop below so they sit at the right point of the
        # in-order Activation instruction stream.
        m_sb = cpool.tile([P, NV, NJ], in_dt)

        def emit_mm2(n, exp_t):
            ups = ps2pool.tile([NJ, 512], f32, tag="ups")
            if USE_BF16:
                for v in range(NV):
                    nc.tensor.matmul(
                        ups[:],
                        lhsT=m_sb[:, v, :],
                        rhs=exp_t[:, v, :],
                        start=(v == 0),
                        stop=(v == NV - 1),
                    )
            else:
                for v in range(0, NV, 2):
                    nc.tensor.matmul(
                        ups[:],
                        lhsT=m_sb[:, v : v + 2, :],
                        rhs=exp_t[:, v : v + 2, :],
                        start=(v == 0),
                        stop=(v == NV - 2),
                        perf_mode=mybir.MatmulPerfMode.DoubleRow,
                    )
            u_sb = upool.tile([NJ, 512], f32, tag="u")
            nc.vector.tensor_copy(u_sb[:], ups[:])
            nc.sync.dma_start(out[:, n, :], u_sb[:])

        pending = None  # (n, exp_t) whose MM2 is deferred one tile
        for n in range(NB):
            et_a, et_b = et_tiles[n]
            exp_t = xpool.tile([P, NV, 512], in_dt, tag="exp")
            for v in range(NV):
                ps = pspool.tile([P, 512], f32, tag="ps")
                if USE_BF16:
                    for k in range(NK):
                        rhs = et_a[:, k, :] if k < 4 else et_b[:, k - 4, :]
                        nc.tensor.matmul(
                            ps[:],
                            lhsT=wt_sb[:, v, k],
                            rhs=rhs,
                            start=(k == 0),
                            stop=(k == NK - 1),
                        )
                else:
                    for k in range(0, NK, 2):
                        rhs = (
                            et_a[:, k : k + 2, :]
                            if k < 4
                            else et_b[:, k - 4 : k - 2, :]
                        )
                        nc.tensor.matmul(
                            ps[:],
                            lhsT=wt_sb[:, v, k : k + 2],
                            rhs=rhs,
                            start=(k == 0),
                            stop=(k == NK - 2),
                            perf_mode=mybir.MatmulPerfMode.DoubleRow,
                        )
                nc.scalar.activation(
                    exp_t[:, v, :],
                    ps[:],
                    Exp,
                    bias=b_sb[:, v : v + 1],
                    scale=exp_scale,
                )
                if n == 0:
                    # mid-stream scalar-engine DMA triggers (see above)
                    if v == 1:
                        nc.scalar.dma_start(m_sb[:], mm[:])
                    elif v == 3:
                        load_et(3, nc.scalar)
                    elif v == 5:
                        load_et(5, nc.scalar)
                    elif v == 7:
                        load_et(7, nc.scalar)
                if pending is not None and v == 1:
                    emit_mm2(*pending)
                    pending = None
            pending = (n, exp_t)
        emit_mm2(*pending)
    _split_excess_waits(nc)
    return nc


def _install_ntff_hook():
    """bass_utils' axon trace path imports antenv.axon_hooks, absent in this
    image; shim it using trn_boot's ctypes NTFF hook."""
    if "antenv.axon_hooks" in sys.modules:
        return
    try:
        from trn_agent_boot.trn_boot import _ntff_profile_via_ctypes

        hook = _ntff_profile_via_ctypes("/opt/axon/libaxon_pjrt.so")
    except Exception:
        hook = None
    mod = types.ModuleType("antenv.axon_hooks")
    mod.get_axon_ntff_profile_hook = lambda: hook
    sys.modules["antenv.axon_hooks"] = mod


def kernel(embeddings, W, b, valid_states):
    global LAST_EXEC_NS, LAST_RESULT
    E = np.asarray(embeddings, dtype=np.float32)
    Wf = np.asarray(W, dtype=np.float32)
    bf = np.asarray(b, dtype=np.float32)
    vs = np.asarray(valid_states).astype(np.int64)

    bf16 = ml_dtypes.bfloat16
    if USE_BF16:
        in_dt = bf16
        Wp = Wf
        w_scale = 1.0
    else:
        in_dt = ml_dtypes.float8_e4m3
        w_scale = _pick_w_scale(float(np.abs(Wf).max()))
        Wp = Wf * w_scale

    # et[n, p, k, j] = E[n*512+j, k*128+p]
    Et = E.T.astype(in_dt)  # [D, B]
    et_host = np.ascontiguousarray(Et.reshape(NK, P, NB, 512).transpose(2, 1, 0, 3))

    # One-hot segment matrix M [N_VALID, 48]
    M = np.zeros((N_VALID, NJ), dtype=in_dt)
    stride = N_TOTAL
    for c, n_i in enumerate(OUTCOMES):
        stride //= n_i
        digit = (vs // stride) % n_i
        M[np.arange(N_VALID), c * 8 + digit] = 1

    in_maps = []
    for core in range(N_CORES):
        sl = slice(core * V_S, (core + 1) * V_S)
        wt_host = np.ascontiguousarray(
            Wp[sl, :].T.astype(in_dt).reshape(NK, P, NV, P).transpose(1, 2, 0, 3)
        )
        m_host = np.ascontiguousarray(M[sl].reshape(NV, P, NJ).transpose(1, 0, 2))
        b_host = np.ascontiguousarray(bf[sl].reshape(NV, P).T)
        in_maps.append({"wt": wt_host, "et": et_host, "mm": m_host, "bias": b_host})

    from concourse.bass_utils import run_bass_kernel_spmd

    key = (USE_BF16, w_scale)
    if key not in _compiled_cache:
        _compiled_cache[key] = _build_nc(w_scale)
    nc_mod = _compiled_cache[key]

    kwargs = {}
    if os.environ.get("KERNEL_TRACE"):
        _install_ntff_hook()
        kwargs["trace"] = True

    res = run_bass_kernel_spmd(
        nc_mod, in_maps, core_ids=list(range(N_CORES)), **kwargs
    )
    LAST_EXEC_NS = res.exec_time_ns
    LAST_RESULT = res

    U = np.zeros((NJ, B), dtype=np.float64)
    for r in res.results:
        U += r["out"].reshape(NJ, B).astype(np.float64)
    denom = U[0:8].sum(axis=0)  # [B] total softmax denominator
    marg = U.reshape(6, 8, B) / denom  # [6, 8, B]
    return np.ascontiguousarray(marg.transpose(0, 2, 1)).astype(np.float32)


# revision 13
# speedup vs baseline: 1.0310x; 1.0161x over previous
"""AdmissibleStatesHead on 8 Trainium2 NeuronCores.

marginals[c] = segment_sum(softmax(E @ W.T + b), digit_c)  ==  P @ M_c
where M is a one-hot [N_VALID, 48] matrix built on host from valid_states.

Device work per core (valid-states sharded 8 ways, batch replicated):
  logits^T tile [128v, 512b] = sum_k wt[k,v].T @ et[k,b]   (fp8 DoubleRow PE, fp32 PSUM)
  exp tile = Exp(logits^T * inv_scale + bias)              (ScalarE, PSUM -> SBUF fp8)
  U^T [48, 512b] += M_chunk.T @ exp_tile                   (fp8 DoubleRow PE)
Host: sum per-core partials, normalize by concept-0 bucket sum (= softmax
denominator), reshape to [6, B, 8]. W is pre-scaled by a power of two into
fp8's range; the Exp activation's free affine undoes it.

Schedule notes (from trace analysis): the PE matmul stream is gapless at
216ns/MM once the clock gate opens (~5.45us after first PE activity), so the
wins are at the edges: (1) input DMA triggers cost ~0.6us of engine time each
and are split across both HWDGE engines (SP + Activation) so the first real
matmul's data lands ~7.7us instead of ~8.4; (2) the clock-gate warm-up runs
tiny N=128 matmuls on uninitialized SBUF (no memset dependency) and hands off
to real matmuls as soon as DMA lands, instead of grinding 10 fixed N=512
throwaways; (3) the last batch tile's final exp/MM2/copy are split into
column halves across engines to shorten the serial drain tail.

Set KERNEL_BF16=1 for a bf16 fallback (~1.8x slower, ~15x more accurate);
KERNEL_TRACE=1 captures an NTFF profile and fills LAST_EXEC_NS.
"""

import os
import sys
import types

import numpy as np
import ml_dtypes

OUTCOMES = [8, 8, 8, 8, 8, 8]
N_TOTAL = 262144
N_VALID = 8192
B, D = 4096, 1024
N_CORES = 8
P = 128
V_S = N_VALID // N_CORES  # 1024 valid states per core
NK = D // P               # 8 contraction chunks
NV = V_S // P             # 8 v-tiles per core
NB = B // 512             # 8 batch tiles of 512
NJ = 48                   # 6 concepts x 8 outcomes

# W values are small (~N(0, 0.02^2) per spec); scale into fp8e4m3's normal
# range and undo the scale for free inside the Exp activation. Chosen per
# call from the data as a power of two; the compiled module is cached per
# scale value.
DEFAULT_W_SCALE = 64.0

USE_BF16 = bool(os.environ.get("KERNEL_BF16"))
N_WARM = int(os.environ.get("KERNEL_WARM", "34"))

LAST_EXEC_NS = None
LAST_RESULT = None
_compiled_cache = {}


def _pick_w_scale(wmax):
    import math

    if not np.isfinite(wmax) or wmax <= 0:
        return DEFAULT_W_SCALE
    # keep max|W*scale| around <=192 (fp8e4m3 max 448), scale a power of 2
    s = 2.0 ** math.floor(math.log2(192.0 / wmax))
    return float(min(max(s, 2.0 ** -10), 2.0 ** 20))


def _split_excess_waits(nc, limit=1):
    """This walrus build rejects instructions carrying more than ~1 sync-wait
    ("Too many sync wait commands"). Hoist excess waits onto injected NoOps
    right before the instruction on the same engine — sequencers are in-order,
    so the semantics are identical."""
    import concourse.mybir as mybir

    ctr = 0
    main_bb = nc.m.functions[0].blocks[0]
    stripped = []
    for ins in main_bb.instructions:
        nm = str(ins.name)
        op = ins.concise_opcode()
        if op == "Drain" or (op == "EventSemaphore" and nm.startswith("barrier_")):
            continue
        stripped.append(ins)
    main_bb.instructions = stripped
    for fn in nc.m.functions:
        for bb in fn.blocks:
            insts = bb.instructions
            new = []
            changed = False
            for ins in insts:
                si = ins.sync_info
                lim = 1 if ins.concise_opcode() == "Drain" else limit
                if si is not None and len(si.on_wait) > lim:
                    waits = list(si.on_wait)
                    for w in waits[:-lim]:
                        ctr += 1
                        nop = mybir.InstNoOp(name=f"waitsplit_{ctr}", ins=[], outs=[])
                        nop.engine = ins.engine
                        nop.sync_info = mybir.SyncInfo(on_update=[], on_wait=[w])
                        new.append(nop)
                    ins.sync_info = mybir.SyncInfo(
                        on_update=list(si.on_update), on_wait=waits[-lim:]
                    )
                    changed = True
                new.append(ins)
            if changed:
                bb.instructions = new


def _patch_tile_tail():
    import concourse.tile as tile
    from concourse.vector_clock import ScopedClock

    if getattr(tile.TileContext, "_tail_patched", False):
        return

    def _drain_and_barrier(self, tick_clock, wait_clock):
        drain_inst = self.nc.sync.drain()
        wait_clock.add_sem_waits(
            drain_inst.ins, ScopedClock({None: tick_clock.global_clock})
        )
        self.nc.all_engine_barrier()
        popped = self.nc._tile_sem_poison_stack.pop()
        assert popped is self._sem_poison
        # No tail clear_and_free_semaphores: the next execution's preamble
        # (dma_reset + sem_clear over the kernel sem range) re-clears them;
        # return the handles to the pool without emitting clear instructions.
        sems = list(self.sems.allocated().values())
        if sems:
            import concourse.bass as bass

            nums = [
                s.num if isinstance(s, bass.SemaphoreHandle) else s for s in sems
            ]
            self.nc._state.prepend_free_semaphores(nums)

    tile.TileContext._drain_and_barrier = _drain_and_barrier
    tile.TileContext._tail_patched = True


def _build_nc(w_scale):
    import concourse.bass as bass
    import concourse.mybir as mybir
    import concourse.tile as tile

    _patch_tile_tail()

    f32 = mybir.dt.float32
    bf16 = mybir.dt.bfloat16
    fp8 = mybir.dt.float8e4
    Exp = mybir.ActivationFunctionType.Exp

    in_dt = bf16 if USE_BF16 else fp8
    exp_scale = 1.0 if USE_BF16 else 1.0 / w_scale

    nc = bass.Bass()
    wt = nc.dram_tensor("wt", [P, NV, NK, P], in_dt, kind="ExternalInput")
    et = nc.dram_tensor("et", [NB, P, NK, 512], in_dt, kind="ExternalInput")
    mm = nc.dram_tensor("mm", [P, NV, NJ], in_dt, kind="ExternalInput")
    bias = nc.dram_tensor("bias", [P, NV], f32, kind="ExternalInput")
    out = nc.dram_tensor("out", [NJ, NB, 512], f32, kind="ExternalOutput")

    # Clock-gate warm-up scratch: read uninitialized SBUF (fp8 garbage is
    # numerically irrelevant — the warm psum is never read) so the PE can
    # start the instant its preamble ends, with no memset dependency.
    warm_g = nc.alloc_sbuf_tensor("warm_garbage", [P, P], in_dt)
    warm_a = nc.alloc_sbuf_tensor("warm_act_out", [P, 16], in_dt)

    with (
        tile.TileContext(nc) as tc,
        tc.tile_pool(name="const", bufs=1) as cpool,
        tc.tile_pool(name="etp", bufs=8) as epool,
        tc.tile_pool(name="expp", bufs=3) as xpool,
        tc.tile_pool(name="ps", bufs=6, space="PSUM") as pspool,
        tc.tile_pool(name="ps2", bufs=2, space="PSUM") as ps2pool,
        tc.tile_pool(name="uo", bufs=2) as upool,
    ):
        # PE HAM warm-up: the clock gate only opens after ~5.4us of sustained
        # PE activity (measured: first-MM + 5.45us). Run tiny N=128 matmuls on
        # garbage SBUF from the earliest possible instant; the real matmul
        # stream takes over as soon as its DMA lands (~7.7us) and rides the
        # ramp to 2.4 GHz.
        warm_ps = pspool.tile([P, 512], f32, tag="ps")
        for _ in range(N_WARM):
            nc.tensor.matmul(
                warm_ps[:, 0:P],
                lhsT=warm_g[:],
                rhs=warm_g[:],
                start=True,
                stop=True,
            )

        # Input DMA: each HWDGE queue (SP=sync, Activation=scalar) starts
        # transferring ~1-1.5us after its first trigger and the two queues
        # contend at ~250 GB/s aggregate early on, so each queue's FIRST
        # transfers are exactly the first-matmul critical set, in need order.
        # scalar q: wt v0:2, et0 k0:4, wt v6:8, m.  sync q: bias, et0 k4:8,
        # wt v2:4, wt v4:6, et1/2/4/6.  (et3/5/7 trigger mid-loop.)
        wt_sb = cpool.tile([P, NV, NK, P], in_dt)
        b_sb = cpool.tile([P, NV], f32)
        et0a = cpool.tile([P, 4, 512], in_dt)
        et0b = cpool.tile([P, 4, 512], in_dt)
        nc.scalar.dma_start(wt_sb[:, 0:2], wt[:, 0:2])
        nc.sync.dma_start(b_sb[:], bias[:])
        nc.scalar.dma_start(et0a[:], et[0][:, 0:4])
        nc.sync.dma_start(et0b[:], et[0][:, 4:8])
        nc.scalar.dma_start(wt_sb[:, 6:8], wt[:, 6:8])
        nc.sync.dma_start(wt_sb[:, 2:4], wt[:, 2:4])
        nc.sync.dma_start(wt_sb[:, 4:6], wt[:, 4:6])
        # Exp table load (~1.3us) on the scalar engine, after its DMA
        # triggers but before the first real activation needs it.
        nc.scalar.activation(warm_a[:], warm_g[:, 0:16], Exp)

        et_tiles = {}

        def load_et(n, engine):
            t = epool.tile([P, NK, 512], in_dt, tag="et", name="et_t")
            engine.dma_start(t[:], et[n])
            et_tiles[n] = (t[:, 0:4], t[:, 4:8])

        et_tiles[0] = (et0a[:], et0b[:])
        m_sb = cpool.tile([P, NV, NJ], in_dt)
        nc.scalar.dma_start(m_sb[:], mm[:])
        load_et(1, nc.sync)
        load_et(2, nc.sync)
        load_et(4, nc.sync)
        load_et(6, nc.sync)

        def emit_mm2(n, exp_t):
            ups = ps2pool.tile([NJ, 512], f32, tag="ups")
            if USE_BF16:
                for v in range(NV):
                    nc.tensor.matmul(
                        ups[:],
                        lhsT=m_sb[:, v, :],
                        rhs=exp_t[:, v, :],
                        start=(v == 0),
                        stop=(v == NV - 1),
                    )
            else:
                for v in range(0, NV, 2):
                    nc.tensor.matmul(
                        ups[:],
                        lhsT=m_sb[:, v : v + 2, :],
                        rhs=exp_t[:, v : v + 2, :],
                        start=(v == 0),
                        stop=(v == NV - 2),
                        perf_mode=mybir.MatmulPerfMode.DoubleRow,
                    )
            u_sb = upool.tile([NJ, 512], f32, tag="u")
            nc.vector.tensor_copy(u_sb[:], ups[:])
            nc.sync.dma_start(out[:, n, :], u_sb[:])

        pending = None  # (n, exp_t) whose MM2 is deferred one tile
        for n in range(NB):
            et_a, et_b = et_tiles[n]
            exp_t = xpool.tile([P, NV, 512], in_dt, tag="exp")
            for v in range(NV):
                ps = pspool.tile([P, 512], f32, tag="ps")
                if USE_BF16:
                    for k in range(NK):
                        rhs = et_a[:, k, :] if k < 4 else et_b[:, k - 4, :]
                        nc.tensor.matmul(
                            ps[:],
                            lhsT=wt_sb[:, v, k],
                            rhs=rhs,
                            start=(k == 0),
                            stop=(k == NK - 1),
                        )
                else:
                    for k in range(0, NK, 2):
                        rhs = (
                            et_a[:, k : k + 2, :]
                            if k < 4
                            else et_b[:, k - 4 : k - 2, :]
                        )
                        nc.tensor.matmul(
                            ps[:],
                            lhsT=wt_sb[:, v, k : k + 2],
                            rhs=rhs,
                            start=(k == 0),
                            stop=(k == NK - 2),
                            perf_mode=mybir.MatmulPerfMode.DoubleRow,
                        )
                nc.scalar.activation(
                    exp_t[:, v, :],
                    ps[:],
                    Exp,
                    bias=b_sb[:, v : v + 1],
                    scale=exp_scale,
                )
                if n == 0:
                    # mid-stream scalar-engine DMA triggers for the odd et
                    # tiles, interleaved with n=0's activations
                    if v == 3:
                        load_et(3, nc.scalar)
                    elif v == 5:
                        load_et(5, nc.scalar)
                    elif v == 7:
                        load_et(7, nc.scalar)
                if pending is not None and v == 1:
                    emit_mm2(*pending)
                    pending = None
            pending = (n, exp_t)
        emit_mm2(*pending)
    _split_excess_waits(nc)
    return nc


def _install_ntff_hook():
    """bass_utils' axon trace path imports antenv.axon_hooks, absent in this
    image; shim it using trn_boot's ctypes NTFF hook."""
    if "antenv.axon_hooks" in sys.modules:
        return
    try:
        from trn_agent_boot.trn_boot import _ntff_profile_via_ctypes

        hook = _ntff_profile_via_ctypes("/opt/axon/libaxon_pjrt.so")
    except Exception:
        hook = None
    mod = types.ModuleType("antenv.axon_hooks")
    mod.get_axon_ntff_profile_hook = lambda: hook
    sys.modules["antenv.axon_hooks"] = mod


def kernel(embeddings, W, b, valid_states):
    global LAST_EXEC_NS, LAST_RESULT
    E = np.asarray(embeddings, dtype=np.float32)
    Wf = np.asarray(W, dtype=np.float32)
    bf = np.asarray(b, dtype=np.float32)
    vs = np.asarray(valid_states).astype(np.int64)

    bf16 = ml_dtypes.bfloat16
    if USE_BF16:
        in_dt = bf16
        Wp = Wf
        w_scale = 1.0
    else:
        in_dt = ml_dtypes.float8_e4m3
        w_scale = _pick_w_scale(float(np.abs(Wf).max()))
        Wp = Wf * w_scale

    # et[n, p, k, j] = E[n*512+j, k*128+p]
    Et = E.T.astype(in_dt)  # [D, B]
    et_host = np.ascontiguousarray(Et.reshape(NK, P, NB, 512).transpose(2, 1, 0, 3))

    # One-hot segment matrix M [N_VALID, 48]
    M = np.zeros((N_VALID, NJ), dtype=in_dt)
    stride = N_TOTAL
    for c, n_i in enumerate(OUTCOMES):
        stride //= n_i
        digit = (vs // stride) % n_i
        M[np.arange(N_VALID), c * 8 + digit] = 1

    in_maps = []
    for core in range(N_CORES):
        sl = slice(core * V_S, (core + 1) * V_S)
        wt_host = np.ascontiguousarray(
            Wp[sl, :].T.astype(in_dt).reshape(NK, P, NV, P).transpose(1, 2, 0, 3)
        )
        m_host = np.ascontiguousarray(M[sl].reshape(NV, P, NJ).transpose(1, 0, 2))
        b_host = np.ascontiguousarray(bf[sl].reshape(NV, P).T)
        in_maps.append({"wt": wt_host, "et": et_host, "mm": m_host, "bias": b_host})

    from concourse.bass_utils import run_bass_kernel_spmd

    key = (USE_BF16, w_scale)
    if key not in _compiled_cache:
        _compiled_cache[key] = _build_nc(w_scale)
    nc_mod = _compiled_cache[key]

    kwargs = {}
    if os.environ.get("KERNEL_TRACE"):
        _install_ntff_hook()
        kwargs["trace"] = True

    res = run_bass_kernel_spmd(
        nc_mod, in_maps, core_ids=list(range(N_CORES)), **kwargs
    )
    LAST_EXEC_NS = res.exec_time_ns
    LAST_RESULT = res

    U = np.zeros((NJ, B), dtype=np.float64)
    for r in res.results:
        U += r["out"].reshape(NJ, B).astype(np.float64)
    denom = U[0:8].sum(axis=0)  # [B] total softmax denominator
    marg = U.reshape(6, 8, B) / denom  # [6, 8, B]
    return np.ascontiguousarray(marg.transpose(0, 2, 1)).astype(np.float32)
